# revision 2
# baseline (speedup 1.0000x reference)
"""Trainium2 Bass kernel for the spiking conv encoder (nn_Encoder_15410342658418).

Shapes (hardcoded): spike [8,2,128,128,32] -> out [8,32,64,64,32].
Data-parallel over batch N=8, one sample per NeuronCore.

t-synchronous per-core pipeline (one pass over t=0..31):
  * conv as im2col matmul, 2 matmuls per t (q halves) into persistent PSUM
    accumulators.  The CUBA current filter cur_t = sum_d 0.75^(t-d) z_d is
    folded INTO the PE accumulation: lhsT for step t is w * 0.75^-t, and the
    ACT evacuation applies scale 0.75^t -- so PSUM holds the weighted
    cumulative sum and ACT writes out exactly cur_t.  No scan pass at all.
  * ACT evacuates PSUM -> U[:, t+1, :] with the 0.75^t rescale.
  * DVE runs the LIF voltage recurrence as ONE fused custom op per step:
      u_t = select(u_{t-1} < 1, u_{t-1}, 0) * 0.9 + cur_t   (in place in U)
  * DVE fuses spike extraction + per-channel fractional delay mix into one
    custom op per 4-step group, writing bf16:
      out_t = (1-f)*[u_t >= 1] + f*[u_{t-1} >= 1]
  * out DRAM [8, 128, 4096] bf16 (t-group major); host upcasts to fp32.
"""

import numpy as np

import concourse.bacc as bacc
import concourse.bass_utils as bass_utils
import concourse.tile as tile
from concourse import mybir

# ---- custom DVE op registration (runtime, self-contained) ----
from concourse.dve_spec import Spec, Src0, Src1, C0, C1, Zero, One, select, lower
from concourse import dve_ops as _dve_ops
from concourse.dve_uop import DveOpSpec


def _register_op(name, spec, subdim=False):
    existing = {op.name: op for op in _dve_ops.OPS}
    if name in existing:
        return existing[name]
    shas = {}
    for ver in ("v3", "v4"):
        try:
            shas[ver] = DveOpSpec(name=name, uops=lower(spec, ver=ver)).sha(ver)
        except Exception:
            pass
    op = _dve_ops.DveOp(name, spec, subdim=subdim, uops_sha=shas)
    _dve_ops.OPS.append(op)
    _dve_ops._SUB_OPCODE_FOR_NAME[name] = (
        _dve_ops._CUSTOM_DVE_ROW_BASE + len(_dve_ops.OPS) - 1
    )
    return op


# u_t = select(u_{t-1} < 1, u_{t-1}, 0) * s0 + cur_t
LIF_STEP = _register_op(
    "LIF_STEP_ANT",
    Spec(
        body=select(Src0 < One, Src0, Zero) * C0 + Src1,
        reference=lambda in0, in1, s0, s1, imm2: (
            np.where(in0 < 1.0, in0, 0.0) * s0 + in1
        ).astype(np.float32),
    ),
)

# out_t = select(u_t >= 1, c1, 0) + select(u_{t-1} >= 1, c0, 0)
DELAY_MIX = _register_op(
    "DELAY_MIX_ANT",
    Spec(
        body=select(Src1 >= One, C1, Zero) + select(Src0 >= One, C0, Zero),
        reference=lambda in0, in1, s0, s1, imm2: (
            np.where(in1 >= 1.0, s1, 0.0) + np.where(in0 >= 1.0, s0, 0.0)
        ).astype(np.float32),
    ),
)

N, C, H, W, T = 8, 2, 128, 128, 32
CH = 32
Hp, Wp = 64, 64
CUR_DECAY = 0.25
VOLT_DECAY = 0.1
LEAK = 1.0 - VOLT_DECAY  # 0.9
DECAY = 1.0 - CUR_DECAY  # 0.75
YB = 4
NYG = Hp // YB  # 16 y-groups
K = 72  # contraction rows (kx, c, ky*4+yb)
Q = NYG * Wp  # 1024 state columns
QH = Q // 2
TG = 4  # t-steps per output group
NG = T // TG  # 8 output groups

_COMPILED = None


def _build_program():
    nc = bacc.Bacc("TRN2", target_bir_lowering=False, debug=False, num_devices=N)
    f32 = mybir.dt.float32
    bf16 = mybir.dt.bfloat16

    x_d = nc.dram_tensor("x", [T, K, Q], f32, kind="ExternalInput")
    wall_d = nc.dram_tensor("wall", [K, T * 128], f32, kind="ExternalInput")
    coef_d = nc.dram_tensor("coef", [128, 2], f32, kind="ExternalInput")
    out_d = nc.dram_tensor("out", [NG, 128, TG * Q], bf16, kind="ExternalOutput")

    from contextlib import ExitStack

    with tile.TileContext(nc) as tc, ExitStack() as ctx:
        _kernel_body(ctx, tc, x_d.ap(), wall_d.ap(), coef_d.ap(), out_d.ap())
    nc.compile()
    return nc


def _kernel_body(ctx, tc, x, wall, coef, out):
    nc = tc.nc
    f32 = mybir.dt.float32
    bf16 = mybir.dt.bfloat16
    Act = mybir.ActivationFunctionType

    consts = ctx.enter_context(tc.tile_pool(name="consts", bufs=1))
    xpool = ctx.enter_context(tc.tile_pool(name="xpool", bufs=3))
    upool = ctx.enter_context(tc.tile_pool(name="upool", bufs=1))
    opool = ctx.enter_context(tc.tile_pool(name="opool", bufs=2))
    psump = ctx.enter_context(tc.tile_pool(name="psump", bufs=1, space="PSUM"))

    wall_t = consts.tile([K, T * 128], f32)
    nc.sync.dma_start(out=wall_t, in_=wall)
    coef_t = consts.tile([128, 2], f32)
    nc.sync.dma_start(out=coef_t, in_=coef)
    f_ap = coef_t[:, 0:1]  # f (delay fraction)
    omf_ap = coef_t[:, 1:2]  # 1 - f

    # U[:, s, :]: s=0 zeros (u_{-1}); s=t+1 holds cur_t, overwritten by u_t.
    U = upool.tile([128, (T + 1) * Q], f32)
    U3 = U.rearrange("p (s q) -> p s q", q=Q)
    nc.vector.memset(U3[:, 0, :], 0.0)

    psA = psump.tile([128, QH], f32, name="psA", tag="psA")
    psB = psump.tile([128, QH], f32, name="psB", tag="psB")

    xt = [None] * T

    def load_x(t):
        xt[t] = xpool.tile([K, Q], f32, tag="xt", name=f"x{t}")
        nc.sync.dma_start(out=xt[t], in_=x[t])

    load_x(0)
    load_x(1)

    for t in range(T):
        if t + 2 < T:
            load_x(t + 2)
        wt = wall_t[:, t * 128 : (t + 1) * 128]
        nc.tensor.matmul(
            psA, lhsT=wt, rhs=xt[t][:, 0:QH], start=(t == 0), stop=(t == T - 1)
        )
        nc.tensor.matmul(
            psB, lhsT=wt, rhs=xt[t][:, QH:Q], start=(t == 0), stop=(t == T - 1)
        )
        sc = float(DECAY**t)
        nc.scalar.activation(
            out=U3[:, t + 1, 0:QH], in_=psA, func=Act.Copy, scale=sc
        )
        nc.scalar.activation(
            out=U3[:, t + 1, QH:Q], in_=psB, func=Act.Copy, scale=sc
        )
        # u_t = select(u_{t-1} < 1, u_{t-1}, 0) * 0.9 + cur_t   (in place)
        nc.vector._custom_dve(
            LIF_STEP,
            out=U3[:, t + 1, :],
            in0=U3[:, t, :],
            in1=U3[:, t + 1, :],
            s0=LEAK,
        )
        if t % TG == TG - 1:
            g = t // TG
            ob = opool.tile([128, TG * Q], bf16, tag="ob", name=f"ob{g}")
            nc.vector._custom_dve(
                DELAY_MIX,
                out=ob,
                in0=U[:, g * TG * Q : (g + 1) * TG * Q],
                in1=U[:, (g * TG + 1) * Q : ((g + 1) * TG + 1) * Q],
                s0=f_ap,
                s1=omf_ap,
            )
            nc.sync.dma_start(out=out[g], in_=ob)


def _host_prep(spike, weight_v, weight_g, delay):
    spike = np.asarray(spike, dtype=np.float32)
    weight_v = np.asarray(weight_v, dtype=np.float32)
    weight_g = np.asarray(weight_g, dtype=np.float32)
    delay = np.asarray(delay, dtype=np.float32)

    vnorm = np.sqrt((weight_v * weight_v).sum(axis=(1, 2, 3), keepdims=True))
    wn = (weight_g[:, None, None, None] * weight_v / vnorm).astype(np.float32)

    # lhsT [72, 128]: row kx*24 + c*12 + ky*4 + yb -> col yb*32 + ch
    wblk = np.zeros((K, 128), dtype=np.float32)
    for yb in range(YB):
        for kx in range(3):
            for c in range(C):
                for ky in range(3):
                    row = kx * 24 + c * 12 + ky * 4 + yb
                    wblk[row, yb * 32 : (yb + 1) * 32] = wn[:, c, ky, kx]

    # wall [72, T*128]: step t block = wblk * 0.75^-t
    wall = np.empty((K, T * 128), dtype=np.float32)
    for t in range(T):
        wall[:, t * 128 : (t + 1) * 128] = wblk * np.float32(DECAY ** (-t))

    f = delay.astype(np.float32)
    coef = np.zeros((128, 2), dtype=np.float32)
    for yb in range(YB):
        s = slice(yb * 32, (yb + 1) * 32)
        coef[s, 0] = f
        coef[s, 1] = 1.0 - f

    # im2col, t-major: xrep[n, t, krow, yg*64+x]
    xpad = np.pad(spike, ((0, 0), (0, 0), (1, 0), (1, 0), (0, 0)))
    xrep = np.empty((N, T, K, Q), dtype=np.float32)
    yg8 = 8 * np.arange(NYG)
    for kx in range(3):
        for ky in range(3):
            for yb in range(4):
                rows = 2 * yb + ky + yg8
                # [n, c, yg, x, t]
                blk = xpad[:, :, rows, kx : kx + 2 * Wp : 2, :]
                for c in range(C):
                    row = kx * 24 + c * 12 + ky * 4 + yb
                    xrep[:, :, row, :] = (
                        blk[:, c].transpose(0, 3, 1, 2).reshape(N, T, Q)
                    )
    return xrep, wall, coef


def _host_post(outs):
    # per-core out [NG, 128, TG*Q] bf16 -> [CH, 64, 64, T] fp32
    full = np.empty((N, CH, Hp, Wp, T), dtype=np.float32)
    for n, o in enumerate(outs):
        a = np.asarray(o).astype(np.float32)
        # [tg, (yb,ch), tt, yg, x] -> [ch, yg, yb, x, tg, tt]
        a = a.reshape(NG, YB, CH, TG, NYG, Wp).transpose(2, 4, 1, 5, 0, 3)
        full[n] = a.reshape(CH, Hp, Wp, T)
    return full


def kernel(spike, weight_v, weight_g, delay):
    global _COMPILED
    if _COMPILED is None:
        _COMPILED = _build_program()
    nc = _COMPILED

    xrep, wall, coef = _host_prep(spike, weight_v, weight_g, delay)
    in_maps = [
        {"x": np.ascontiguousarray(xrep[n]), "wall": wall, "coef": coef}
        for n in range(N)
    ]
    res = bass_utils.run_bass_kernel_spmd(nc, in_maps, core_ids=list(range(N)))
    return _host_post([r["out"] for r in res.results])
